# revision 7
# baseline (speedup 1.0000x reference)
"""BigBird attention kernel for 8 Trainium2 NeuronCores.

Head-parallel sharding: core h computes head h end-to-end (QKV projections,
masked attention, and its partial slice of the output projection); the host
sums the 8 partial output projections (the tensor-parallel unshard) and adds
the output bias.

Shapes are hardcoded for B=1, S=4096, C=512, H=8, Dh=64, fp32.

All score/AV matmuls run in the transposed layout S^T[s', q] so that the
attention-weight tensor feeds the PE directly as lhsT/rhs without any
on-chip transposition of P:
    S^T tile  = K_tile       @ Q^T chunk      (lhsT = K^T slice)
    denom     = ones^T        @ P^T tile       (PE column-sum)
    O^T chunk = V_tile^T      @ P^T tile
    partial   = (O^T slice)^T @ Wo_h^T
Softmax skips the max-subtraction (scores are O(1) for sane inputs; masked
entries are exp()'d then zeroed with a predicated copy, so no -inf needed).
"""

import math
import sys

import numpy as np

sys.path.insert(0, "/opt/trn_rl_repo")

B, S, C, H = 1, 4096, 512, 8
DH = C // H  # 64
QC = 512  # q-chunk (moving dim of the score matmuls)
NQ = S // QC  # 8
NT = S // 128  # 32 s'-tiles

_CACHE = {}


def _build_bass():
    import concourse.bass as bass
    import concourse.bacc as bacc
    import concourse.mybir as mybir
    import concourse.tile as tile
    from concourse.masks import make_identity

    f32 = mybir.dt.float32
    f32r = mybir.dt.float32r
    u8 = mybir.dt.uint8

    nc = bacc.Bacc("TRN2", target_bir_lowering=False, debug=False)

    x_d = nc.dram_tensor("x", [S, C], f32, kind="ExternalInput")
    maskz_d = nc.dram_tensor("maskz", [NT, NQ, 128, QC], u8, kind="ExternalInput")
    wqT_d = nc.dram_tensor("wqT", [C, DH], f32, kind="ExternalInput")
    wkT_d = nc.dram_tensor("wkT", [C, DH], f32, kind="ExternalInput")
    wvT_d = nc.dram_tensor("wvT", [C, DH], f32, kind="ExternalInput")
    woT_d = nc.dram_tensor("woT", [DH, C], f32, kind="ExternalInput")
    bq8_d = nc.dram_tensor("bq8", [DH, 1], f32, kind="ExternalInput")
    bk_d = nc.dram_tensor("bk", [DH, 1], f32, kind="ExternalInput")
    bv_d = nc.dram_tensor("bv", [DH, 1], f32, kind="ExternalInput")
    out_d = nc.dram_tensor("partial", [S, C], f32, kind="ExternalOutput")

    with tile.TileContext(nc) as tc:
        with (
            tc.tile_pool(name="const", bufs=1) as cpool,
            tc.tile_pool(name="big", bufs=1) as bigpool,
        ):
            ident = cpool.tile([128, 128], f32)
            make_identity(nc, ident)
            ones = cpool.tile([128, 1], f32)
            nc.vector.memset(ones, 1.0)
            zeros = cpool.tile([128, QC], f32)
            nc.vector.memset(zeros, 0.0)

            wqT = cpool.tile([128, 4, DH], f32)
            wkT = cpool.tile([128, 4, DH], f32)
            wvT = cpool.tile([128, 4, DH], f32)
            nc.sync.dma_start(out=wqT, in_=wqT_d.rearrange("(a p) d -> p a d", p=128))
            nc.sync.dma_start(out=wkT, in_=wkT_d.rearrange("(a p) d -> p a d", p=128))
            nc.sync.dma_start(out=wvT, in_=wvT_d.rearrange("(a p) d -> p a d", p=128))
            woT = cpool.tile([DH, C], f32)
            nc.sync.dma_start(out=woT, in_=woT_d[:, :])
            bq8 = cpool.tile([DH, 1], f32)
            bk_t = cpool.tile([DH, 1], f32)
            bv_t = cpool.tile([DH, 1], f32)
            nc.sync.dma_start(out=bq8, in_=bq8_d[:, :])
            nc.sync.dma_start(out=bk_t, in_=bk_d[:, :])
            nc.sync.dma_start(out=bv_t, in_=bv_d[:, :])

            # big persistent tensors
            xT = bigpool.tile([128, 4, S], f32)  # X^T: [c%128, c//128, s]
            qT = bigpool.tile([DH, S], f32)
            kT = bigpool.tile([DH, S], f32)
            vS = bigpool.tile([128, NT, DH], f32)  # V row-tiles

            # ---- phase 0: load x, build X^T via PE transposes ----
            with (
                tc.tile_pool(name="xload", bufs=3) as xpool,
                tc.tile_pool(name="xps", bufs=3, space="PSUM") as xps,
            ):
                for t in range(NT):
                    xt = xpool.tile([128, C], f32)
                    nc.sync.dma_start(out=xt, in_=x_d[t * 128 : (t + 1) * 128, :])
                    ps = xps.tile([128, 512], f32)
                    for cb in range(4):
                        nc.tensor.transpose(
                            ps[:, cb * 128 : (cb + 1) * 128],
                            xt[:, cb * 128 : (cb + 1) * 128],
                            ident,
                        )
                    nc.vector.tensor_copy(
                        xT[:, :, t * 128 : (t + 1) * 128],
                        ps.rearrange("p (a q) -> p a q", a=4),
                    )

            # ---- phase 1: projections ----
            with (
                tc.tile_pool(name="pjps", bufs=4, space="PSUM") as pjps,
                tc.tile_pool(name="vT", bufs=1) as vtpool,
            ):
                mult = mybir.AluOpType.mult
                add = mybir.AluOpType.add
                vT = vtpool.tile([DH, S], f32)
                for j in range(NQ):
                    sl = slice(j * QC, (j + 1) * QC)
                    for name, wT, bias_ap, dst in (
                        ("q", wqT, bq8, qT),
                        ("k", wkT, bk_t, kT),
                        ("v", wvT, bv_t, vT),
                    ):
                        ps = pjps.tile([DH, QC], f32)
                        for cb in range(4):
                            nc.tensor.matmul(
                                ps,
                                wT[:, cb, :],
                                xT[:, cb, sl],
                                start=(cb == 0),
                                stop=(cb == 3),
                            )
                        if name == "q":
                            nc.vector.tensor_scalar(
                                dst[:, sl], ps, 0.125, bias_ap, op0=mult, op1=add
                            )
                        else:
                            nc.vector.tensor_scalar_add(dst[:, sl], ps, bias_ap)
                # V^T -> V row-tiles
                for t in range(NT):
                    ps = pjps.tile([128, DH], f32)
                    nc.tensor.transpose(
                        ps, vT[:, t * 128 : (t + 1) * 128], ident[:DH, :DH]
                    )
                    nc.vector.tensor_copy(vS[:, t, :], ps)

            # ---- phase 2: attention + output projection, per q-chunk ----
            with (
                tc.tile_pool(name="sps", bufs=2, space="PSUM") as sps,
                tc.tile_pool(name="dps", bufs=1, space="PSUM") as dps,
                tc.tile_pool(name="ops", bufs=2, space="PSUM") as ops,
                tc.tile_pool(name="outps", bufs=2, space="PSUM") as outps,
                tc.tile_pool(name="pt", bufs=3) as ptpool,
                tc.tile_pool(name="mk", bufs=4) as mkpool,
                tc.tile_pool(name="sm", bufs=2) as smpool,
                tc.tile_pool(name="oT", bufs=2) as otpool,
                tc.tile_pool(name="res", bufs=3) as respool,
            ):
                for j in range(NQ):
                    qsl = slice(j * QC, (j + 1) * QC)
                    den_ps = dps.tile([1, QC], f32)
                    o_ps = ops.tile([DH, QC], f32)
                    for t in range(NT):
                        mz = mkpool.tile([128, QC], u8)
                        nc.sync.dma_start(out=mz, in_=maskz_d[t, j])
                        s_ps = sps.tile([128, QC], f32)
                        nc.tensor.matmul(
                            s_ps,
                            kT[:, t * 128 : (t + 1) * 128],
                            qT[:, qsl],
                            start=True,
                            stop=True,
                        )
                        pt = ptpool.tile([128, QC], f32)
                        nc.scalar.activation(
                            pt, s_ps, mybir.ActivationFunctionType.Exp
                        )
                        nc.vector.copy_predicated(pt, mz, zeros)
                        nc.tensor.matmul(
                            den_ps,
                            ones,
                            pt,
                            start=(t == 0),
                            stop=(t == NT - 1),
                        )
                        nc.tensor.matmul(
                            o_ps,
                            vS[:, t, :],
                            pt,
                            start=(t == 0),
                            stop=(t == NT - 1),
                        )
                    oT_sb = otpool.tile([DH, QC], f32)
                    nc.scalar.activation(
                        oT_sb, o_ps, mybir.ActivationFunctionType.Copy
                    )
                    den_sb = smpool.tile([1, QC], f32)
                    nc.vector.reciprocal(den_sb, den_ps)
                    for sub in range(QC // 128):
                        # recip [1,128] -> [128,1] via PE transpose
                        r_ps = dps.tile([128, 1], f32, tag="rps")
                        nc.tensor.transpose(
                            r_ps[:, 0:1],
                            den_sb[:, sub * 128 : (sub + 1) * 128],
                            ident[:1, :1],
                        )
                        r_sb = smpool.tile([128, 1], f32, tag="rsb")
                        nc.vector.tensor_copy(r_sb, r_ps[:, 0:1])
                        p_ps = outps.tile([128, C], f32)
                        nc.tensor.matmul(
                            p_ps,
                            oT_sb[:, sub * 128 : (sub + 1) * 128],
                            woT,
                            start=True,
                            stop=True,
                        )
                        res = respool.tile([128, C], f32)
                        nc.vector.tensor_scalar_mul(res, p_ps, r_sb)
                        nc.sync.dma_start(
                            out=out_d[j * QC + sub * 128 : j * QC + (sub + 1) * 128, :],
                            in_=res,
                        )
    nc.compile()
    return nc


def _get_nc():
    if "nc" not in _CACHE:
        _CACHE["nc"] = _build_bass()
    return _CACHE["nc"]


def _make_in_maps(inp):
    x2 = np.ascontiguousarray(
        np.asarray(inp["x"], dtype=np.float32).reshape(S, C)
    )
    m = np.asarray(inp["attn_mask"])
    # inverted mask (1 where masked-out), tiled [NT, NQ, 128, QC] so every
    # per-tile DMA is one contiguous 64KB read
    mz = (~m).astype(np.uint8).T.reshape(NT, 128, NQ, QC).transpose(0, 2, 1, 3)
    mz = np.ascontiguousarray(mz)
    Wq, Wk, Wv, Wo = (np.asarray(inp[k], np.float32) for k in ("Wq", "Wk", "Wv", "Wo"))
    bq, bk, bv = (np.asarray(inp[k], np.float32) for k in ("bq", "bk", "bv"))
    in_maps = []
    for h in range(H):
        sl = slice(h * DH, (h + 1) * DH)
        in_maps.append(
            {
                "x": x2,
                "maskz": mz,
                "wqT": np.ascontiguousarray(Wq[sl, :].T),
                "wkT": np.ascontiguousarray(Wk[sl, :].T),
                "wvT": np.ascontiguousarray(Wv[sl, :].T),
                "woT": np.ascontiguousarray(Wo[:, sl].T),
                "bq8": bq[sl].reshape(DH, 1) / 8.0,
                "bk": bk[sl].reshape(DH, 1),
                "bv": bv[sl].reshape(DH, 1),
            }
        )
    return in_maps


def kernel(x, attn_mask, Wq, bq, Wk, bk, Wv, bv, Wo, bo):
    from concourse.bass_utils import run_bass_kernel_spmd

    inp = dict(x=x, attn_mask=attn_mask, Wq=Wq, bq=bq, Wk=Wk, bk=bk,
               Wv=Wv, bv=bv, Wo=Wo, bo=bo)
    nc = _get_nc()
    in_maps = _make_in_maps(inp)
    res = run_bass_kernel_spmd(nc, in_maps, core_ids=list(range(H)))
    acc = res.results[0]["partial"].astype(np.float64)
    for c in range(1, H):
        acc += res.results[c]["partial"]
    out = acc.astype(np.float32) + np.asarray(bo, dtype=np.float32)[None, :]
    return out.reshape(B, S, C)


# revision 11
# speedup vs baseline: 1.0323x; 1.0323x over previous
"""BigBird attention kernel for 8 Trainium2 NeuronCores.

Head-parallel sharding: core h computes head h end-to-end (QKV projections,
masked attention, and its partial slice of the output projection); the host
sums the 8 partial output projections (the tensor-parallel unshard) and adds
the output bias.

Shapes are hardcoded for B=1, S=4096, C=512, H=8, Dh=64, fp32.

All score/AV matmuls run in the transposed layout S^T[s', q] so that the
attention-weight tensor feeds the PE directly as lhsT/rhs without any
on-chip transposition of P:
    S^T tile  = K_tile       @ Q^T chunk      (lhsT = K^T slice)
    denom     = ones^T        @ P^T tile       (PE column-sum)
    O^T chunk = V_tile^T      @ P^T tile
    partial   = (O^T slice)^T @ Wo_h^T
Softmax skips the max-subtraction (scores are O(1) for sane inputs; masked
entries are exp()'d then zeroed with a predicated copy, so no -inf needed).
"""

import math
import sys

import numpy as np

sys.path.insert(0, "/opt/trn_rl_repo")

B, S, C, H = 1, 4096, 512, 8
DH = C // H  # 64
QC = 512  # q-chunk (moving dim of the score matmuls)
NQ = S // QC  # 8
NT = S // 128  # 32 s'-tiles

import os

PREC = os.environ.get("BASS_PREC", "f32r")

_CACHE = {}


def _build_bass():
    import concourse.bass as bass
    import concourse.bacc as bacc
    import concourse.mybir as mybir
    import concourse.tile as tile
    from concourse.masks import make_identity

    f32 = mybir.dt.float32
    f32r = mybir.dt.float32r
    u8 = mybir.dt.uint8
    # DT_S: dtype of score/projection matmul inputs; DT_P: dtype of
    # post-softmax matmul inputs (attention weights, V, O, Wo)
    DT_S = f32r if PREC == "f32r" else f32
    DT_P = f32r if PREC in ("f32r", "mixed") else f32

    nc = bacc.Bacc("TRN2", target_bir_lowering=False, debug=False)

    x_d = nc.dram_tensor("x", [S, C], f32, kind="ExternalInput")
    maskz_d = nc.dram_tensor("maskz", [NT, NQ, 128, QC], u8, kind="ExternalInput")
    wqT_d = nc.dram_tensor("wqT", [C, DH], f32, kind="ExternalInput")
    wkT_d = nc.dram_tensor("wkT", [C, DH], f32, kind="ExternalInput")
    wvT_d = nc.dram_tensor("wvT", [C, DH], f32, kind="ExternalInput")
    woT_d = nc.dram_tensor("woT", [DH, C], f32, kind="ExternalInput")
    bq8_d = nc.dram_tensor("bq8", [DH, 1], f32, kind="ExternalInput")
    bk_d = nc.dram_tensor("bk", [DH, 1], f32, kind="ExternalInput")
    bv_d = nc.dram_tensor("bv", [DH, 1], f32, kind="ExternalInput")
    out_d = nc.dram_tensor("partial", [S, C], f32, kind="ExternalOutput")

    with tile.TileContext(nc) as tc:
        with (
            tc.tile_pool(name="const", bufs=1) as cpool,
            tc.tile_pool(name="big", bufs=1) as bigpool,
        ):
            ident = cpool.tile([128, 128], f32)
            make_identity(nc, ident)
            ones_f = cpool.tile([128, 1], f32)
            nc.vector.memset(ones_f, 1.0)
            ones = ones_f
            if DT_P != f32:
                ones = cpool.tile([128, 1], DT_P, tag="ones_r")
                nc.vector.tensor_copy(ones, ones_f)
            neg30 = cpool.tile([128, QC], f32, tag="neg30")
            nc.vector.memset(neg30, -30.0)

            wqT = cpool.tile([128, 4, DH], f32)
            wkT = cpool.tile([128, 4, DH], f32)
            wvT = cpool.tile([128, 4, DH], f32)
            nc.sync.dma_start(out=wqT, in_=wqT_d.rearrange("(a p) d -> p a d", p=128))
            nc.sync.dma_start(out=wkT, in_=wkT_d.rearrange("(a p) d -> p a d", p=128))
            nc.sync.dma_start(out=wvT, in_=wvT_d.rearrange("(a p) d -> p a d", p=128))
            woT0 = cpool.tile([DH, C], f32)
            nc.sync.dma_start(out=woT0, in_=woT_d[:, :])
            woT = woT0
            if DT_P != f32:
                woT = cpool.tile([DH, C], DT_P, tag="woT_r")
                nc.vector.tensor_copy(woT, woT0)
            bq8 = cpool.tile([DH, 1], f32)
            bk_t = cpool.tile([DH, 1], f32)
            bv_t = cpool.tile([DH, 1], f32)
            nc.sync.dma_start(out=bq8, in_=bq8_d[:, :])
            nc.sync.dma_start(out=bk_t, in_=bk_d[:, :])
            nc.sync.dma_start(out=bv_t, in_=bv_d[:, :])

            if DT_S != f32:
                wqT_r = cpool.tile([128, 4, DH], DT_S, tag="wq_r")
                wkT_r = cpool.tile([128, 4, DH], DT_S, tag="wk_r")
                wvT_r = cpool.tile([128, 4, DH], DT_S, tag="wv_r")
                nc.vector.tensor_copy(wqT_r, wqT)
                nc.vector.tensor_copy(wkT_r, wkT)
                nc.vector.tensor_copy(wvT_r, wvT)
                wqT, wkT, wvT = wqT_r, wkT_r, wvT_r

            # big persistent tensors
            xT = bigpool.tile([128, 4, S], DT_S)  # X^T: [c%128, c//128, s]
            qT = bigpool.tile([DH, S], DT_S)
            kT = bigpool.tile([DH, S], DT_S)
            vS = bigpool.tile([128, NT, DH], DT_P)  # V row-tiles

            # ---- phase 0: load x, build X^T via PE transposes ----
            with (
                tc.tile_pool(name="xload", bufs=3) as xpool,
                tc.tile_pool(name="xps", bufs=3, space="PSUM") as xps,
            ):
                for t in range(NT):
                    xt = xpool.tile([128, C], f32)
                    nc.sync.dma_start(out=xt, in_=x_d[t * 128 : (t + 1) * 128, :])
                    ps = xps.tile([128, 512], f32)
                    for cb in range(4):
                        nc.tensor.transpose(
                            ps[:, cb * 128 : (cb + 1) * 128],
                            xt[:, cb * 128 : (cb + 1) * 128],
                            ident,
                        )
                    nc.vector.tensor_copy(
                        xT[:, :, t * 128 : (t + 1) * 128],
                        ps.rearrange("p (a q) -> p a q", a=4),
                    )

            # ---- phase 1: projections ----
            with (
                tc.tile_pool(name="pjps", bufs=4, space="PSUM") as pjps,
                tc.tile_pool(name="vT", bufs=1) as vtpool,
            ):
                mult = mybir.AluOpType.mult
                add = mybir.AluOpType.add
                vT = vtpool.tile([DH, S], f32)
                for j in range(NQ):
                    sl = slice(j * QC, (j + 1) * QC)
                    for name, wT, bias_ap, dst in (
                        ("q", wqT, bq8, qT),
                        ("k", wkT, bk_t, kT),
                        ("v", wvT, bv_t, vT),
                    ):
                        ps = pjps.tile([DH, QC], f32)
                        for cb in range(4):
                            nc.tensor.matmul(
                                ps,
                                wT[:, cb, :],
                                xT[:, cb, sl],
                                start=(cb == 0),
                                stop=(cb == 3),
                            )
                        if name == "q":
                            nc.vector.tensor_scalar(
                                dst[:, sl], ps, 0.125, bias_ap, op0=mult, op1=add
                            )
                        else:
                            nc.vector.tensor_scalar_add(dst[:, sl], ps, bias_ap)
                # V^T -> V row-tiles
                for t in range(NT):
                    ps = pjps.tile([128, DH], f32)
                    nc.tensor.transpose(
                        ps, vT[:, t * 128 : (t + 1) * 128], ident[:DH, :DH]
                    )
                    nc.vector.tensor_copy(vS[:, t, :], ps)

            # ---- phase 2: attention + output projection, per q-chunk ----
            with (
                tc.tile_pool(name="sps", bufs=2, space="PSUM") as sps,
                tc.tile_pool(name="dps", bufs=1, space="PSUM") as dps,
                tc.tile_pool(name="ops", bufs=2, space="PSUM") as ops,
                tc.tile_pool(name="outps", bufs=2, space="PSUM") as outps,
                tc.tile_pool(name="pt", bufs=3) as ptpool,
                tc.tile_pool(name="mk", bufs=4) as mkpool,
                tc.tile_pool(name="sm", bufs=2) as smpool,
                tc.tile_pool(name="oT", bufs=2) as otpool,
                tc.tile_pool(name="res", bufs=3) as respool,
            ):
                for j in range(NQ):
                    qsl = slice(j * QC, (j + 1) * QC)
                    den_ps = dps.tile([1, QC], f32)
                    o_ps = ops.tile([DH, QC], f32)
                    for t in range(NT):
                        mz = mkpool.tile([128, QC], u8)
                        nc.sync.dma_start(out=mz, in_=maskz_d[t, j])
                        s_ps = sps.tile([128, QC], f32)
                        nc.tensor.matmul(
                            s_ps,
                            kT[:, t * 128 : (t + 1) * 128],
                            qT[:, qsl],
                            start=True,
                            stop=True,
                        )
                        nc.vector.copy_predicated(s_ps, mz, neg30)
                        pt = ptpool.tile([128, QC], DT_P)
                        nc.scalar.activation(
                            pt, s_ps, mybir.ActivationFunctionType.Exp
                        )
                        nc.tensor.matmul(
                            den_ps,
                            ones,
                            pt,
                            start=(t == 0),
                            stop=(t == NT - 1),
                        )
                        nc.tensor.matmul(
                            o_ps,
                            vS[:, t, :],
                            pt,
                            start=(t == 0),
                            stop=(t == NT - 1),
                        )
                    oT_sb = otpool.tile([DH, QC], DT_P)
                    nc.scalar.activation(
                        oT_sb, o_ps, mybir.ActivationFunctionType.Copy
                    )
                    den_sb = smpool.tile([1, QC], f32)
                    nc.vector.reciprocal(den_sb, den_ps)
                    for sub in range(QC // 128):
                        # recip [1,128] -> [128,1] via PE transpose
                        r_ps = dps.tile([128, 1], f32, tag="rps")
                        nc.tensor.transpose(
                            r_ps[:, 0:1],
                            den_sb[:, sub * 128 : (sub + 1) * 128],
                            ident[:1, :1],
                        )
                        r_sb = smpool.tile([128, 1], f32, tag="rsb")
                        nc.vector.tensor_copy(r_sb, r_ps[:, 0:1])
                        p_ps = outps.tile([128, C], f32)
                        nc.tensor.matmul(
                            p_ps,
                            oT_sb[:, sub * 128 : (sub + 1) * 128],
                            woT,
                            start=True,
                            stop=True,
                        )
                        res = respool.tile([128, C], f32)
                        nc.vector.tensor_scalar_mul(res, p_ps, r_sb)
                        nc.sync.dma_start(
                            out=out_d[j * QC + sub * 128 : j * QC + (sub + 1) * 128, :],
                            in_=res,
                        )
    nc.compile()
    return nc


def _get_nc():
    if "nc" not in _CACHE:
        _CACHE["nc"] = _build_bass()
    return _CACHE["nc"]


def _make_in_maps(inp):
    x2 = np.ascontiguousarray(
        np.asarray(inp["x"], dtype=np.float32).reshape(S, C)
    )
    m = np.asarray(inp["attn_mask"])
    # inverted mask (1 where masked-out), tiled [NT, NQ, 128, QC] so every
    # per-tile DMA is one contiguous 64KB read
    mz = (~m).astype(np.uint8).T.reshape(NT, 128, NQ, QC).transpose(0, 2, 1, 3)
    mz = np.ascontiguousarray(mz)
    Wq, Wk, Wv, Wo = (np.asarray(inp[k], np.float32) for k in ("Wq", "Wk", "Wv", "Wo"))
    bq, bk, bv = (np.asarray(inp[k], np.float32) for k in ("bq", "bk", "bv"))
    in_maps = []
    for h in range(H):
        sl = slice(h * DH, (h + 1) * DH)
        in_maps.append(
            {
                "x": x2,
                "maskz": mz,
                "wqT": np.ascontiguousarray(Wq[sl, :].T),
                "wkT": np.ascontiguousarray(Wk[sl, :].T),
                "wvT": np.ascontiguousarray(Wv[sl, :].T),
                "woT": np.ascontiguousarray(Wo[:, sl].T),
                "bq8": bq[sl].reshape(DH, 1) / 8.0,
                "bk": bk[sl].reshape(DH, 1),
                "bv": bv[sl].reshape(DH, 1),
            }
        )
    return in_maps


def kernel(x, attn_mask, Wq, bq, Wk, bk, Wv, bv, Wo, bo):
    from concourse.bass_utils import run_bass_kernel_spmd

    inp = dict(x=x, attn_mask=attn_mask, Wq=Wq, bq=bq, Wk=Wk, bk=bk,
               Wv=Wv, bv=bv, Wo=Wo, bo=bo)
    nc = _get_nc()
    in_maps = _make_in_maps(inp)
    res = run_bass_kernel_spmd(nc, in_maps, core_ids=list(range(H)))
    acc = res.results[0]["partial"].astype(np.float64)
    for c in range(1, H):
        acc += res.results[c]["partial"]
    out = acc.astype(np.float32) + np.asarray(bo, dtype=np.float32)[None, :]
    return out.reshape(B, S, C)
